# revision 41
# baseline (speedup 1.0000x reference)
"""Trainium2 Bass kernel: per-voxel eigenvalues of 3x3 symmetric matrices.

Input  X: (2, 9, 96, 96, 96) float32 -- each voxel holds a row-major 3x3
matrix in the channel dim.  Output: (2, 3, 96, 96, 96) float32, the
eigenvalues of the symmetrized matrix, ascending in the channel dim.

Strategy: embarrassingly parallel over voxels.  The 884736 voxels per batch
are sharded 8 ways (110592 = 128 partitions x 864 free per core per batch).
Each core runs a straight-line closed-form trigonometric eigensolver:

    q   = tr/3,  aq/bq/cq = diag - q,  D/E/F = doubled off-diagonals
    p2  = ||A - qI||_F^2 = aq^2+bq^2+cq^2 + (D^2+E^2+F^2)/2
    det = det(A - qI)    = aq*bq*cq + (D*E*F - aq*F^2 - bq*E^2 - cq*D^2)/4
    r   = det / (2 p^3),  p = sqrt(p2/6)
    asin(r) = atan( sqrt(54)*det / sqrt(p2^3 - 54 det^2) )
    lambda_k = q + 2p * sin(-asin(r)/3 + {2pi/3, 0, 4pi/3})

Roots are computed as exp(k*ln(x)) on the scalar engine (the Rsqrt/Reciprocal
activations are banned and vector reciprocal is 8 cyc/elem).  Only two ACT
table sets are used: natural_log_exp_and_others, then trig_and_small.

Work is split across the three elementwise-capable engines (Vector / GPSIMD /
Scalar) so no single engine is the bottleneck.
"""

import sys

if "/opt/trn_rl_repo" not in sys.path:
    sys.path.insert(0, "/opt/trn_rl_repo")

import math

import numpy as np

N_CORES = 8
B = 2
DHW = 96 * 96 * 96          # 884736 voxels per batch
PER = DHW // N_CORES        # 110592 voxels per batch per core
P = 128                     # SBUF partitions
FB = PER // P               # 864 free elems per batch per core
FT = B * FB                 # packed free dim per core (both batches)
# Chunk sizes along the packed free dim (sum = FT).  Uneven on purpose:
# the last chunk is small so its serial tail exposes little latency.
CHUNKS = [864, 864]
NCHUNK = len(CHUNKS)
TP = max(CHUNKS)            # max tile free dim (slot sizing)
SQ_ON_POOL = False          # route ddq/eeq/ffq squares to GPSIMD, q to DVE

SQRT2 = math.sqrt(2.0)
E1_BIAS = 0.5 * math.log(54.0) + 1.5 * math.log(2.0)
P2_BIAS = 0.5 * math.log(2.0 / 3.0) - 0.5 * math.log(2.0)
LN_EPS = 1e-20
TWO_PI_3 = 2.0 * math.pi / 3.0
PI_3 = math.pi / 3.0
R2_CLAMP = 1.0 - 2.0 ** -23

_CACHE = {}


def _build(split_waits=True, nrep=1):
    import contextlib

    import concourse.bass as bass
    import concourse.tile as tile
    from concourse import mybir

    fp32 = mybir.dt.float32
    AF = mybir.ActivationFunctionType

    nc = bass.Bass("TRN2", target_bir_lowering=False, debug=False,
                   num_devices=N_CORES)
    x = nc.dram_tensor("x", [9, P, FT], fp32, kind="ExternalInput").ap()
    y = nc.dram_tensor("y", [3, P, FT], fp32, kind="ExternalOutput").ap()

    # Activation biases must exist as SBUF const APs before use.
    for cval in (E1_BIAS, P2_BIAS, LN_EPS, TWO_PI_3, PI_3):
        cval = float(cval)
        if (fp32, cval) not in nc.const_aps.aps:
            ctens = nc.alloc_sbuf_tensor(f"const-f32-{cval}", [128, 1], fp32)
            nc.gpsimd.memset(ctens.ap(), cval)
            nc.const_aps.aps[(fp32, cval)] = ctens.ap()
    nc.all_engine_barrier()

    V, G, S = nc.vector, nc.gpsimd, nc.scalar

    with tile.TileContext(nc) as tc:
        with tc.tile_pool(name="sl", bufs=1) as pool:
            n_slots = 24
            free_slots = list(range(n_slots))
            name2slot = {}
            tiles = {}

            cur_tp = [TP]

            def alloc(name):
                s = free_slots.pop(0)
                name2slot[name] = s
                t = pool.tile([P, cur_tp[0]], fp32, tag=f"s{s}")
                tiles[name] = t
                return t

            def rel(*names):
                for name in names:
                    free_slots.append(name2slot.pop(name))
                    del tiles[name]

            def tt(eng, dst, a, b, op):
                d = alloc(dst)
                fn = {"add": eng.tensor_add, "sub": eng.tensor_sub,
                      "mul": eng.tensor_mul}[op]
                fn(d[:, :], tiles[a][:, :], tiles[b][:, :])
                return d

            act_insts = {}

            def act(dst, src, func, scale=1.0, bias=0.0):
                d = alloc(dst)
                inst = S.activation(d[:, :], tiles[src][:, :], func,
                                    bias=float(bias), scale=float(scale))
                act_insts[(cur_key[0], dst)] = inst
                return d

            def ts_max(eng, dst, src, const):
                d = alloc(dst)
                eng.tensor_scalar_max(d[:, :], tiles[src][:, :], float(const))
                return d

            def ts_min(eng, dst, src, const):
                d = alloc(dst)
                eng.tensor_scalar_min(d[:, :], tiles[src][:, :], float(const))
                return d

            cur_key = [0]
            for rep in range(nrep):
              coff = 0
              for ci in range(NCHUNK):
                # Disjoint slot set per chunk parity: cross-chunk slot reuse
                # creates false WAR deps that serialize the pipeline.
                par = (rep * NCHUNK + ci) % 2
                free_slots[:] = [par * n_slots + s for s in range(n_slots)]
                cur_key[0] = (rep, ci)
                cur_tp[0] = CHUNKS[ci]
                sl2 = slice(coff, coff + CHUNKS[ci])
                coff += CHUNKS[ci]

                # ---- load the 9 channel planes (both batches side by side,
                # one DMA per channel so readers wait on one DMA sem each)
                for ch in (1, 3, 0, 4, 8, 2, 6, 5, 7):
                    t = alloc(f"x{ch}")
                    nc.sync.dma_start(out=t[:, :], in_=x[ch][:, sl2])

                # ---- linear stage
                tt(G, "D", "x1", "x3", "add")
                tt(G, "E", "x2", "x6", "add")
                tt(G, "F", "x5", "x7", "add")
                rel("x1", "x3", "x2", "x6", "x5", "x7")
                tt(V, "t0", "x0", "x4", "add")
                tt(V, "tr", "t0", "x8", "add")
                if SQ_ON_POOL:
                    dq = alloc("q")
                    V.tensor_scalar_mul(dq[:, :], tiles["tr"][:, :], 1.0 / 3.0)
                else:
                    act("q", "tr", AF.Copy, scale=1.0 / 3.0)
                rel("t0", "tr")
                tt(V, "aq", "x0", "q", "sub")
                tt(V, "bq", "x4", "q", "sub")
                tt(V, "cq", "x8", "q", "sub")
                rel("x0", "x4", "x8")

                # ---- squares; a2/b2/c2 carry a factor 2 so the p2 sum
                # needs no 0.5 rescale (we track p2x = 2*p2)
                if SQ_ON_POOL:
                    tt(G, "ddq", "D", "D", "mul")
                    tt(G, "eeq", "E", "E", "mul")
                    tt(G, "ffq", "F", "F", "mul")
                else:
                    act("ddq", "D", AF.Square)
                    act("eeq", "E", AF.Square)
                    act("ffq", "F", AF.Square)
                act("a2", "aq", AF.Square, scale=SQRT2)
                act("b2", "bq", AF.Square, scale=SQRT2)
                act("c2", "cq", AF.Square, scale=SQRT2)

                # ---- det(A - qI)
                tt(G, "w1", "D", "E", "mul")
                tt(G, "w2", "w1", "F", "mul")
                rel("D", "E", "F", "w1")
                tt(V, "u1", "bq", "cq", "mul")
                tt(V, "G1", "aq", "u1", "mul")
                rel("u1")
                tt(V, "v1", "aq", "ffq", "mul")
                tt(V, "v2", "cq", "ddq", "mul")
                tt(G, "v3", "bq", "eeq", "mul")
                rel("aq", "bq", "cq")
                tt(V, "v4", "v1", "v2", "add")
                tt(V, "v5", "v4", "v3", "add")
                rel("v1", "v2", "v3", "v4")
                tt(V, "y1", "w2", "v5", "sub")
                rel("w2", "v5")
                d_ = alloc("y1q")
                V.tensor_scalar_mul(d_[:, :], tiles["y1"][:, :], 0.25)
                rel("y1")
                tt(V, "det", "G1", "y1q", "add")
                rel("G1", "y1q")

                # ---- p2x = 2*||A - qI||_F^2 (a2/b2/c2 pre-doubled)
                tt(G, "s1", "ddq", "eeq", "add")
                tt(G, "p1", "s1", "ffq", "add")
                rel("ddq", "eeq", "ffq", "s1")
                tt(V, "s2", "a2", "b2", "add")
                tt(V, "s3", "s2", "c2", "add")
                rel("a2", "b2", "c2", "s2")
                tt(V, "p2x", "s3", "p1", "add")
                rel("s3", "p1")

                # ---- r = det/(2p^3), asin(r) = 2*atan(r/(1+sqrt(1-r^2)))
                act("lnp2", "p2x", AF.Ln, bias=LN_EPS)             # ln(2 p2)
                rel("p2x")
                act("e1", "lnp2", AF.Exp, scale=-1.5, bias=E1_BIAS)
                tt(V, "rr", "det", "e1", "mul")                    # r
                rel("det", "e1")
                act("r2", "rr", AF.Square)
                ts_min(V, "r2c", "r2", R2_CLAMP)
                rel("r2")
                act("ln1mr2", "r2c", AF.Ln, scale=-1.0, bias=1.0)  # ln(1-r^2)
                rel("r2c")
                act("s", "ln1mr2", AF.Exp, scale=0.5)              # sqrt(1-r^2)
                rel("ln1mr2")
                act("ln1s", "s", AF.Ln, bias=1.0)                  # ln(1+s)
                rel("s")
                act("inv1s", "ln1s", AF.Exp, scale=-1.0)           # 1/(1+s)
                rel("ln1s")
                tt(G, "t2", "rr", "inv1s", "mul")                  # in [-1, 1]
                rel("rr", "inv1s")
                act("at", "t2", AF.Arctan)                         # asin(r)/2
                rel("t2")
                act("P2", "lnp2", AF.Exp, scale=0.5, bias=P2_BIAS)  # 2p
                rel("lnp2")
                act("c1", "at", AF.Sin, scale=-2.0 / 3.0, bias=TWO_PI_3)
                act("c2n", "at", AF.Sin, scale=-2.0 / 3.0, bias=PI_3)
                act("c3", "at", AF.Sin, scale=-2.0 / 3.0)
                rel("at")
                tt(V, "m1", "P2", "c1", "mul")
                tt(V, "lmax", "q", "m1", "add")
                rel("c1", "m1")
                tt(G, "m2", "P2", "c2n", "mul")
                tt(G, "lmin", "q", "m2", "sub")                    # q - P2*c2n
                rel("c2n", "m2")
                tt(V, "m3", "P2", "c3", "mul")
                tt(V, "lmid", "q", "m3", "add")
                rel("c3", "m3", "P2", "q")

                # ---- store ascending eigenvalues
                for k, name in enumerate(("lmin", "lmid", "lmax")):
                    nc.sync.dma_start(out=y[k][:, sl2], in_=tiles[name][:, :])
                rel("lmin", "lmid", "lmax")

            # Group ACT ops by table set across chunks: delay each chunk's
            # first trig-set op until the next chunk's last ln/exp-set op,
            # so the whole kernel needs only 2 ACT table loads.
            if NCHUNK > 1:
                from concourse.bass import _add_dep_helper
                for rep in range(nrep):
                    for ci in range(NCHUNK - 1):
                        a = act_insts.get(((rep, ci), "at"))
                        b = act_insts.get(((rep, ci + 1), "inv1s"))
                        if a is not None and b is not None:
                            _add_dep_helper(a.ins, b.ins, sync=False,
                                            reason="act-table-grouping")

    if split_waits:
        _split_multi_waits(nc, mybir)
    return nc


def _split_multi_waits(nc, mybir):
    """walrus codegen allows a single sync-wait slot per TPB instruction;
    hoist extra waits onto standalone NoOps on the same engine."""
    for f in nc.m.functions:
        for blk in f.blocks:
            il = blk.instructions
            i = 0
            while i < len(il):
                inst = il[i]
                si = inst.sync_info
                if si is not None and si.on_wait and len(si.on_wait) > 1:
                    waits = list(si.on_wait)
                    for w in waits[:-1]:
                        nop = mybir.InstNoOp(
                            name=nc.get_next_instruction_name(),
                            engine=inst.engine,
                            ins=[],
                            outs=[],
                            sync_info=mybir.SyncInfo(on_wait=[w], on_update=[]),
                            bass_nofuse=True,
                        )
                        il.insert(i, nop)
                        i += 1
                    si.on_wait = waits[-1:]
                i += 1


def get_program():
    if "nc" not in _CACHE:
        _CACHE["nc"] = _build()
    return _CACHE["nc"]


def shard_inputs(X):
    """X: (2,9,96,96,96) float32 -> list of per-core {"x": (9,128,1728)}."""
    x = np.asarray(X, dtype=np.float32).reshape(B, 9, DHW)
    maps = []
    for c in range(N_CORES):
        # (B, 9, PER) -> (9, P, B, FB) -> (9, P, FT)
        slab = x[:, :, c * PER:(c + 1) * PER].reshape(B, 9, P, FB)
        xc = np.ascontiguousarray(slab.transpose(1, 2, 0, 3)).reshape(9, P, FT)
        maps.append({"x": xc})
    return maps


def unshard_outputs(results):
    out = np.empty((B, 3, DHW), dtype=np.float32)
    for c, r in enumerate(results):
        yc = np.asarray(r["y"]).reshape(3, P, B, FB).transpose(2, 0, 1, 3)
        out[:, :, c * PER:(c + 1) * PER] = yc.reshape(B, 3, PER)
    return out.reshape(B, 3, 96, 96, 96)


def kernel(X):
    from concourse.bass_utils import run_bass_kernel_spmd

    nc = get_program()
    in_maps = shard_inputs(np.asarray(X))
    res = run_bass_kernel_spmd(nc, in_maps, list(range(N_CORES)))
    return unshard_outputs(res.results)
